# revision 5
# baseline (speedup 1.0000x reference)
"""VQ codebook kernel for Trainium2, 8-core data-parallel over batch.

Problem (hardcoded): z [64,256,32,32] f32, proj_w [128,256], proj_b [128],
embed [1024,128].  Per core: 8 batches.

Algorithm per core (exact f32 association order to match the jax reference):
  zp   = proj_w @ z[b] + b            (PE, [D=128 part, HW=1024])
  s    = sum_d zp^2                   (ones-matmul, [1, 1024])
  dist = (s - 2 z.e) + E              (PSUM accumulate order: s, -2M, E)
  scan = prefix-min(dist)             (DVE tensor_tensor_scan)
  k*   = sum_t sign(scan_t - m)       (ACT Sign + accum_out, m = scan[-1])
  z_q  = embed[k*]                    (indirect DMA gather)
  rep  = z_q^T                        (PE transpose -> [D, HW] output layout)
"""

import os
import sys

import numpy as np

if "/opt/trn_rl_repo" not in sys.path:
    sys.path.insert(0, "/opt/trn_rl_repo")

_CACHE = {}

NB = 8      # batches per core
C = 256
D = 128
HW = 1024   # 32*32
K = 1024
NT = 8      # 128-row n-tiles per batch


def _build_program():
    from contextlib import ExitStack

    import concourse.bass as bass
    import concourse.mybir as mybir
    from concourse import bacc
    from concourse.masks import make_identity
    from concourse.tile import TileContext

    F32 = mybir.dt.float32
    I32 = mybir.dt.int32
    AF = mybir.ActivationFunctionType
    OP = mybir.AluOpType

    nc = bacc.Bacc("TRN2", target_bir_lowering=False, debug=False)

    z = nc.dram_tensor("z", [NB, C, HW], F32, kind="ExternalInput")
    pw = nc.dram_tensor("proj_w", [D, C], F32, kind="ExternalInput")
    pb = nc.dram_tensor("proj_b", [D, 1], F32, kind="ExternalInput")
    emb = nc.dram_tensor("embed", [K, D], F32, kind="ExternalInput")
    rep = nc.dram_tensor("rep", [NB, D, HW], F32, kind="ExternalOutput")
    idx_out = nc.dram_tensor("min_idx", [NB, NT, 128], I32, kind="ExternalOutput")

    with TileContext(nc) as tc, ExitStack() as ctx:
        const = ctx.enter_context(tc.tile_pool(name="const", bufs=1))
        zpool = ctx.enter_context(tc.tile_pool(name="zin", bufs=2))
        zppool = ctx.enter_context(tc.tile_pool(name="zp", bufs=2))
        scanp = ctx.enter_context(tc.tile_pool(name="scan", bufs=2))
        small = ctx.enter_context(tc.tile_pool(name="small", bufs=4))
        zqp = ctx.enter_context(tc.tile_pool(name="zq", bufs=3))
        psum_d = ctx.enter_context(tc.tile_pool(name="pd", bufs=2, space="PSUM"))
        psum_a = ctx.enter_context(tc.tile_pool(name="pa", bufs=1, space="PSUM"))
        psum_t = ctx.enter_context(tc.tile_pool(name="pt", bufs=2, space="PSUM"))

        # ---------- constants / preamble ----------
        identity = const.tile([128, 128], F32)
        make_identity(nc, identity)
        ones_col = const.tile([128, 1], F32)
        nc.vector.memset(ones_col, 1.0)
        ones_row = const.tile([1, 512], F32)
        nc.vector.memset(ones_row, 1.0)
        b_col = const.tile([D, 1], F32)
        nc.sync.dma_start(out=b_col, in_=pb[:, :])
        pw_sb = const.tile([D, C], F32)
        nc.sync.dma_start(out=pw_sb, in_=pw[:, :])

        # wT: [c-part (chunk), d] so lhsT chunks contract over c
        wT = const.tile([128, 256], F32)
        for cc in range(2):
            pt = psum_t.tile([128, 128], F32, tag="t")
            nc.tensor.transpose(
                out=pt, in_=pw_sb[:, cc * 128 : (cc + 1) * 128], identity=identity
            )
            nc.scalar.activation(
                out=wT[:, cc * 128 : (cc + 1) * 128], in_=pt, func=AF.Copy
            )

        # em2T = -2 * embed^T  [D part, K]
        em2T = const.tile([128, K], F32)
        for kc in range(8):
            et = zqp.tile([128, 128], F32, tag="zq")
            nc.sync.dma_start(out=et, in_=emb[kc * 128 : (kc + 1) * 128, :])
            pt = psum_t.tile([128, 128], F32, tag="t")
            nc.tensor.transpose(out=pt, in_=et, identity=identity)
            nc.scalar.activation(
                out=em2T[:, kc * 128 : (kc + 1) * 128], in_=pt, func=AF.Copy,
                scale=-2.0,
            )

        # E_row = sum_d e^2 as [1, K]:  Square(em2T) = 4 e^2, ones-matmul, x0.25
        embsq = zppool.tile([128, K], F32, tag="zpsq")
        nc.scalar.activation(out=embsq, in_=em2T, func=AF.Square)
        psE = psum_d.tile([1, K], F32, tag="d")
        for ch in range(2):
            nc.tensor.matmul(
                out=psE[:, ch * 512 : (ch + 1) * 512],
                lhsT=ones_col,
                rhs=embsq[:, ch * 512 : (ch + 1) * 512],
                start=True,
                stop=True,
            )
        E_row = const.tile([1, K], F32)
        nc.scalar.activation(out=E_row, in_=psE, func=AF.Copy, scale=0.25)

        # ---------- per-batch ----------
        for b in range(NB):
            z_sb = zpool.tile([128, 2 * HW], F32)
            for cc in range(2):
                nc.sync.dma_start(
                    out=z_sb[:, cc * HW : (cc + 1) * HW],
                    in_=z[b, cc * 128 : (cc + 1) * 128, :],
                )

            # zp = wT.T @ z (+bias later)   [D, HW]
            ps_zp = psum_a.tile([128, HW], F32, tag="zp")
            for cc in range(2):
                for hc in range(2):
                    nc.tensor.matmul(
                        out=ps_zp[:, hc * 512 : (hc + 1) * 512],
                        lhsT=wT[:, cc * 128 : (cc + 1) * 128],
                        rhs=z_sb[:, cc * HW + hc * 512 : cc * HW + (hc + 1) * 512],
                        start=(cc == 0),
                        stop=(cc == 1),
                    )
            zp_sb = zppool.tile([128, HW], F32, tag="zp")
            nc.scalar.activation(
                out=zp_sb, in_=ps_zp, func=AF.Identity, bias=b_col[:, 0:1]
            )
            zpsq = zppool.tile([128, HW], F32, tag="zpsq")
            nc.scalar.activation(out=zpsq, in_=zp_sb, func=AF.Square)

            # s = ones^T @ zpsq  [1, HW]
            ps_s = psum_d.tile([1, HW], F32, tag="d")
            for ch in range(2):
                nc.tensor.matmul(
                    out=ps_s[:, ch * 512 : (ch + 1) * 512],
                    lhsT=ones_col,
                    rhs=zpsq[:, ch * 512 : (ch + 1) * 512],
                    start=True,
                    stop=True,
                )
            s_row = small.tile([1, HW], F32, tag="srow")
            nc.scalar.activation(out=s_row, in_=ps_s, func=AF.Copy)

            idxcol = small.tile([128, NT], F32, tag="idxcol")

            for j in range(NT):
                jsl = slice(j * 128, (j + 1) * 128)
                ps_d = psum_d.tile([128, K], F32, tag="d")
                # -2M first (multi-pass internally but self-contained), then
                # rank-1 s and rank-1 E: each a single exact fp32 add, giving
                # fl(fl(-2M + s) + E) == reference fl(fl(s - 2M) + E).
                for ch in range(2):
                    sl = slice(ch * 512, (ch + 1) * 512)
                    nc.tensor.matmul(
                        out=ps_d[:, sl], lhsT=zp_sb[:, jsl], rhs=em2T[:, sl],
                        start=True, stop=False, skip_group_check=True,
                    )
                for ch in range(2):
                    sl = slice(ch * 512, (ch + 1) * 512)
                    nc.tensor.matmul(
                        out=ps_d[:, sl], lhsT=s_row[:, jsl], rhs=ones_row,
                        start=False, stop=False, skip_group_check=True,
                    )
                for ch in range(2):
                    sl = slice(ch * 512, (ch + 1) * 512)
                    nc.tensor.matmul(
                        out=ps_d[:, sl], lhsT=ones_row[:, 0:128], rhs=E_row[:, sl],
                        start=False, stop=True, skip_group_check=True,
                    )

                # prefix-min scan along k
                scan_sb = scanp.tile([128, K], F32, tag="scan")
                nc.vector.tensor_tensor_scan(
                    out=scan_sb,
                    data0=ps_d,
                    data1=ones_col.to_broadcast([128, K]),
                    initial=3.0e38,
                    op0=OP.min,
                    op1=OP.bypass,
                )
                # k* = sum_t -sign(m - scan_t) via ACT accumulate
                trash = scanp.tile([128, K], F32, tag="trash")
                nc.scalar.activation(
                    out=trash,
                    in_=scan_sb,
                    func=AF.Sign,
                    scale=-1.0,
                    bias=scan_sb[:, K - 1 : K],
                    accum_out=idxcol[:, j : j + 1],
                )
                idx_i32 = small.tile([128, 1], I32, tag="idxi")
                nc.vector.tensor_scalar(
                    out=idx_i32, in0=idxcol[:, j : j + 1], scalar1=-1.0,
                    scalar2=None, op0=OP.mult,
                )
                # gather z_q rows, transpose to [D, n]
                zq_sb = zqp.tile([128, 128], F32, tag="zq")
                nc.gpsimd.indirect_dma_start(
                    out=zq_sb,
                    out_offset=None,
                    in_=emb[:, :],
                    in_offset=bass.IndirectOffsetOnAxis(ap=idx_i32[:, 0:1], axis=0),
                )
                ps_t = psum_t.tile([128, 128], F32, tag="t")
                nc.tensor.transpose(out=ps_t, in_=zq_sb, identity=identity)
                zqT = zqp.tile([128, 128], F32, tag="zqT")
                nc.scalar.activation(out=zqT, in_=ps_t, func=AF.Copy)
                nc.sync.dma_start(out=rep[b, :, jsl], in_=zqT)

            # indices out: transpose [128, NT] -> [NT, 128], negate+cast, DMA
            ps_i = psum_t.tile([NT, 128], F32, tag="t")
            nc.tensor.transpose(out=ps_i, in_=idxcol, identity=identity)
            idxT = small.tile([NT, 128], I32, tag="idxT")
            nc.vector.tensor_scalar(
                out=idxT, in0=ps_i, scalar1=-1.0, scalar2=None, op0=OP.mult
            )
            nc.sync.dma_start(out=idx_out[b], in_=idxT)

    nc.compile()
    return nc


def _get_program():
    if "nc" not in _CACHE:
        _CACHE["nc"] = _build_program()
    return _CACHE["nc"]


def run_sharded(z, proj_w, proj_b, embed, trace=False):
    """Run the SPMD kernel; returns (rep [64,128,1024], idx [65536], results obj)."""
    from concourse.bass_utils import run_bass_kernel_spmd

    nc = _get_program()
    z = np.ascontiguousarray(z, dtype=np.float32).reshape(64, C, HW)
    pw = np.ascontiguousarray(proj_w, dtype=np.float32)
    pb = np.ascontiguousarray(proj_b, dtype=np.float32).reshape(D, 1)
    emb = np.ascontiguousarray(embed, dtype=np.float32)

    in_maps = []
    for core in range(8):
        in_maps.append(
            {
                "z": np.ascontiguousarray(z[core * NB : (core + 1) * NB]),
                "proj_w": pw,
                "proj_b": pb,
                "embed": emb,
            }
        )
    res = run_bass_kernel_spmd(nc, in_maps, list(range(8)), trace=trace)
    reps = []
    idxs = []
    for core in range(8):
        out = res.results[core]
        reps.append(np.asarray(out["rep"]))
        idxs.append(np.asarray(out["min_idx"]).reshape(-1))
    rep_full = np.concatenate(reps, axis=0)  # [64, 128, 1024]
    idx_full = np.concatenate(idxs, axis=0)  # [65536]
    return rep_full, idx_full, res


def kernel(z, proj_w, proj_b, embed):
    rep_full, idx_full, _ = run_sharded(z, proj_w, proj_b, embed, trace=False)
    rep_out = rep_full.reshape(64, D, 32, 32).astype(np.float32)
    return rep_out, idx_full.astype(np.int32)
